# revision 15
# baseline (speedup 1.0000x reference)
"""Differentiating1D kernel for Trainium2 (Bass/Tile), 8-core data parallel.

Problem: x (16, 8192, 512) f32; y[:, t] = x[:, t+1] - x[:, t] for t < L-1,
y[:, L-1] = y[:, L-2]  (last diff repeated). Pure memory-bound.

Sharding: batch dim 16 -> 2 batches per core. Per core the shard is viewed
as (16384, 512) rows. Rows are laid out along SBUF partitions in
contiguous blocks of 128 rows per partition (row r = 128*p + k), so the
step-diff becomes a *within-partition* shifted subtract on the free axis:
HBM is read exactly once. Per-partition boundary rows (last row of each
partition needs the next partition's first row) come from one small
strided aux load; batch-end rows (8191, 16383) are recomputed as
duplicates of the previous diff.

Pipeline: loads on the SP HWDGE ring, stores on the ACT HWDGE ring,
triple-buffered tiles. Each chunk's bulk store is issued right after the
main subtract (split_store) so stores are not gated on the next chunk's
load; small first/last chunks shorten pipeline fill and drain.
"""

import sys

import numpy as np

try:
    import concourse  # noqa: F401
except ImportError:  # pragma: no cover
    for _p in ("/opt/trn_rl_repo", "/root/.axon_site/_ro/trn_rl_repo"):
        if _p not in sys.path:
            sys.path.insert(0, _p)

import concourse.bass as bass
import concourse.tile as tile
from concourse import bacc, mybir
from concourse.bass_utils import run_bass_kernel_spmd

B, L, F = 16, 8192, 512
N_CORES = 8
BPC = B // N_CORES          # batches per core = 2
R = BPC * L                 # rows per core = 16384
P = 128                     # SBUF partitions
K = R // P                  # rows per partition = 128
DT = mybir.dt.float32

# partitions whose last row (k = K-1) is a batch-end row -> duplicate fix
_BATCH_END_PARTS = sorted((b * L + L - 1) // K for b in range(BPC))  # [63, 127]

_NC_CACHE = {}


def _build(reps=1, chunks=None, bufs=3, in_place=False, serialize_reps=False,
           split_store=True, aux_on_act=True):
    """reps>1 repeats the full pass back-to-back in one NEFF (identical
    output each rep) — used only for slope-based HW timing in test.py.
    serialize_reps puts an all-engine barrier between reps so the slope
    measures the full single-pass span (incl. pipeline fill/drain).
    chunks: per-chunk row counts (sum = K); small edge chunks shorten
    pipeline fill and drain."""
    if chunks is None:
        chunks = [8] + [16] * 7 + [8]
    if isinstance(chunks, int):
        assert K % chunks == 0
        chunks = [chunks] * (K // chunks)
    assert sum(chunks) == K, chunks
    assert all(c >= 2 for c in chunks)
    nchunk = len(chunks)
    kmax = max(chunks)
    starts = [sum(chunks[:j]) for j in range(nchunk)]  # row offset of chunk j
    in_bufs, out_bufs = bufs if isinstance(bufs, tuple) else (bufs, bufs)

    nc = bacc.Bacc(
        "TRN2", target_bir_lowering=False, debug=False, num_devices=N_CORES
    )
    x = nc.dram_tensor("x", [R, F], DT, kind="ExternalInput")
    y = nc.dram_tensor("y", [R, F], DT, kind="ExternalOutput")
    x3 = x.ap().rearrange("(p k) f -> p (k f)", p=P)   # [128, K*F]
    y3 = y.ap().rearrange("(p k) f -> p (k f)", p=P)
    # aux[p] = x[128*(p+1)]  (first row of the next partition), p = 0..126
    aux_src = bass.AP(x, P * F, [[P * F, P - 1], [1, F]])

    with tile.TileContext(nc) as tc:
        with (
            tc.tile_pool(name="inp", bufs=in_bufs) as inp,
            tc.tile_pool(name="outp", bufs=out_bufs) as outp,
            tc.tile_pool(name="auxp", bufs=1) as auxp,
        ):
            aux = auxp.tile([P, F], DT)
            # p = P-1 is never loaded (no next partition); zero it so the
            # full-width subtract below reads initialized data.
            nc.vector.memset(aux[:], 0)

            def load(j):
                cf = chunks[j] * F
                o = starts[j] * F
                t = inp.tile([P, kmax * F], DT, tag="in")
                nc.sync.dma_start(out=t[:, 0:cf], in_=x3[:, o:o + cf])
                return t

            for _rep in range(reps):
                if serialize_reps and _rep:
                    tc.strict_bb_all_engine_barrier()
                cur = load(0)
                # aux load on the store (ACT) ring, which is idle during
                # pipeline fill; only the last chunk consumes aux.
                if _rep == 0:
                    eng = nc.scalar if aux_on_act else nc.sync
                    eng.dma_start(out=aux[0:P - 1, :], in_=aux_src)
                for j in range(nchunk):
                    cf = chunks[j] * F
                    o = starts[j] * F
                    nxt = load(j + 1) if j + 1 < nchunk else None
                    if in_place:
                        yt = cur
                    else:
                        yt = outp.tile([P, kmax * F], DT, tag="out")
                    # rows k = 0..kc-2 of this chunk: diff within the chunk
                    nc.vector.tensor_sub(
                        yt[:, 0:cf - F], cur[:, F:cf], cur[:, 0:cf - F]
                    )
                    if split_store:
                        # bulk store gated only on the main subtract, not on
                        # the next chunk's load / boundary subtract
                        nc.scalar.dma_start(
                            out=y3[:, o:o + cf - F], in_=yt[:, 0:cf - F]
                        )
                    if nxt is not None:
                        # last row of chunk: first row of next chunk - last
                        nc.vector.tensor_sub(
                            yt[:, cf - F:cf], nxt[:, 0:F], cur[:, cf - F:cf]
                        )
                    else:
                        # last chunk: last row of partition p needs partition
                        # p+1's first row (aux). Batch-end partitions get
                        # garbage in this subtract (DVE ops can't anchor at
                        # partition 63/127); their final row is stored from
                        # the previous diff row instead (duplicate), via
                        # partition-split small stores below.
                        nc.vector.tensor_sub(
                            yt[:, cf - F:cf], aux[:, :], cur[:, cf - F:cf]
                        )
                    if not split_store and nxt is not None:
                        nc.scalar.dma_start(
                            out=y3[:, o:o + cf], in_=yt[:, 0:cf]
                        )
                        cur = nxt
                        continue
                    if nxt is not None:
                        nc.scalar.dma_start(
                            out=y3[:, o + cf - F:o + cf], in_=yt[:, cf - F:cf]
                        )
                    else:
                        if not split_store:
                            nc.scalar.dma_start(
                                out=y3[:, o:o + cf - F], in_=yt[:, 0:cf - F]
                            )
                        # final row: batch-end partitions store the previous
                        # diff row (duplicate); others the aux-based diff.
                        lo = 0
                        for pe in _BATCH_END_PARTS + [None]:
                            hi = P if pe is None else pe
                            if hi > lo:
                                nc.scalar.dma_start(
                                    out=bass.AP(
                                        y,
                                        (lo * K + K - 1) * F,
                                        [[K * F, hi - lo], [1, F]],
                                    ),
                                    in_=yt[lo:hi, cf - F:cf],
                                )
                            if pe is not None:
                                nc.scalar.dma_start(
                                    out=bass.AP(y, (pe * K + K - 1) * F,
                                                [[K * F, 1], [1, F]]),
                                    in_=yt[pe:pe + 1, cf - 2 * F:cf - F],
                                )
                            lo = hi if pe is None else pe + 1
                    cur = nxt

    nc.compile()
    return nc


def _get_nc():
    if "nc" not in _NC_CACHE:
        _NC_CACHE["nc"] = _build()
    return _NC_CACHE["nc"]


def _run(x, trace=False, **spmd_kwargs):
    """Returns (out, BassKernelResults)."""
    x = np.asarray(x, dtype=np.float32)
    assert x.shape == (B, L, F), x.shape
    nc = _get_nc()
    in_maps = [
        {"x": np.ascontiguousarray(x[i * BPC:(i + 1) * BPC].reshape(R, F))}
        for i in range(N_CORES)
    ]
    res = run_bass_kernel_spmd(
        nc, in_maps, list(range(N_CORES)), trace=trace, **spmd_kwargs
    )
    out = np.concatenate(
        [np.asarray(r["y"]).reshape(BPC, L, F) for r in res.results], axis=0
    )
    return out, res


def kernel(x: np.ndarray) -> np.ndarray:
    out, _ = _run(x, trace=False)
    return out
